# revision 17
# baseline (speedup 1.0000x reference)
"""DLRM DotInteraction kernel for Trainium2 (Bass/Tile), 8-core data parallel.

Problem: dense_feature [B=16384, D=128] f32, sparse_stack [S=26, B, D] f32.
cat = [dense; sparse] per sample -> [B, N=27, D]; G_b = cat_b @ cat_b^T;
out = [dense | tril(G_b) (378 vals, row-major incl diag)] -> [B, 506] f32.

Per core (B_c = 2048 samples = 16 tiles of 128), default config ("stream"):
  1. SWDGE cast-DMA loads f32 HBM -> f16 SBUF, natural layout nat[s, j, d].
  2. TensorE f16 transpose of each feature slab -> PSUM, DVE/ACT copy to
     SBUF xt[d, j, s].
  3. TensorE Gram per sample: 4 col-tiled matmuls per group (tile_position
     (0,32c)), K=128, M=N=27, f16 in, f32 PSUM [32c+i, q, j].
  4. Full-width ACT copy PSUM -> gcol[(c,i) part, (j, g)] f16 per tile.
  5. DVE 32x32 StreamTranspose swaps (i<->g) within 32-blocks: lands
     samples on partitions (strans[(c,g) part, tp, j, i]) with NO DMA.
  6. Tril-compaction: 27 engine copies per supertile (f16->f32 cast fused),
     then one contiguous 259 KB store per tile (split over both HWDGE rings).
"""

import numpy as np

import concourse.bacc as bacc
import concourse.mybir as mybir
import concourse.tile as tile
from concourse import bass_utils
from concourse.masks import make_identity

B = 16384
D = 128
S = 26
N = S + 1  # 27
NCORES = 8
BC = B // NCORES  # 2048 samples per core
PT = 128  # samples per sbuf tile
GPR = 16  # groups per psum round
TRI = N * (N + 1) // 2  # 378
W = D + TRI  # 506
TPS = 8  # tiles per supertile

f32 = mybir.dt.float32
f16 = mybir.dt.float16


def build_kernel(
    b_core: int = BC,
    reps: int = 1,
    *,
    tps: int = 4,
    flatten: str = "stream",  # "stream" | "rowq" | "hbm" (see below)
    trmode: str = "f16",  # "f32" (PE f32 transpose) | "f16" (cast then f16 transpose)
    nat_bufs: int = 4,
    xt_bufs: int = 4,
    gcol_bufs: int = 4,
    upto: str = "full",  # "load" | "xt" | "gram" | "flat" | "full" (bench ablation)
    flat_split: int = 2,  # flatten DMA queue split: i % flat_split == 0 -> scalar
    store_split: int = 2,  # store queue split: tp % store_split == 0 -> scalar
    shared_lhsT: bool = False,  # timing probe: all gram matmuls share one lhsT
    dense_via_rowq: bool = False,  # f32 mode: dense loaded once (into rowq),
    # j=0 transpose reads rowq; saves the duplicate dense HBM read
    row_bufs: int = 2,
    gcol_f16: bool = False,  # gcol+tri in f16, engine-cast to f32 before store
    big_store: bool = False,  # one store DMA per supertile instead of per tile
    psum_bufs: int = 4,
    psumt_bufs: int = 4,
    gram_layout: str = "qj",  # "qj" (baseline) | "jq" (strided MM out; contiguous drain)
    drain_eng: str = "act",  # gram psum->gcol drain engine: "act" | "dve"
    rowq_eng: str = "sync",  # rowq dense-load DMA queue: "sync" | "gpsimd"
    mm_cols: int = 1,  # timing probe (mm mode only): rhs spans this many samples
    xt_reps: int = 1,  # timing probe: issue the transpose stage this many times
):
    nc = bacc.Bacc("TRN2", target_bir_lowering=False, debug=False)
    dense = nc.dram_tensor("dense", [b_core, D], f32, kind="ExternalInput").ap()
    sparse = nc.dram_tensor("sparse", [S, b_core, D], f32, kind="ExternalInput").ap()
    out = nc.dram_tensor("out", [b_core, W], f32, kind="ExternalOutput").ap()

    t_total = b_core // PT
    gpt = PT // 4  # 32 groups per tile
    rpt = gpt // GPR  # psum rounds per tile
    tps = min(tps, t_total)
    n_super = t_total // tps

    with tile.TileContext(nc) as tc:
        with (
            tc.tile_pool(name="singles", bufs=1) as singles,
            tc.tile_pool(name="nat", bufs=nat_bufs) as nat_pool,
            tc.tile_pool(name="xt", bufs=xt_bufs) as xt_pool,
            tc.tile_pool(name="gcol", bufs=gcol_bufs) as gcol_pool,
            tc.tile_pool(name="row", bufs=row_bufs) as row_pool,
            tc.tile_pool(name="psum", bufs=psum_bufs, space="PSUM") as psum_pool,
            tc.tile_pool(name="psumt", bufs=psumt_bufs, space="PSUM") as psumt_pool,
        ):
            id_dt = f32 if trmode == "f32" else f16
            ident = singles.tile([128, 128], id_dt, name="ident")
            make_identity(nc, ident)

            for _rep in range(reps):
                if flatten == "hbm":
                    # dense passthrough: single HBM->HBM DMA
                    nc.scalar.dma_start(out=out[:, 0:D], in_=dense[:, :])
                for st in range(n_super):
                    # gcol[32c+i, g, tp, j] = Gram[i,j] of sample 32c+g in
                    # tile tp of this supertile.
                    gdt = f16 if gcol_f16 else f32
                    if flatten not in ("rect", "stream") and upto not in ("load", "xt", "mm"):
                        gcol = gcol_pool.tile([128, gpt, tps, N], gdt)
                    if flatten == "stream" and upto in ("flat", "full"):
                        rowq = row_pool.tile([128, tps, W], f32)
                        strans = row_pool.tile([128, tps, N, 32], f16, tag="strans")
                    elif flatten == "rowq":
                        rowq = row_pool.tile([128, tps, W], f32)
                        if gcol_f16:
                            rowt = row_pool.tile([128, tps, TRI], f16, tag="rowt")
                    elif flatten == "rect":
                        rowq = row_pool.tile([128, tps, W], f32)
                        rowt27 = row_pool.tile([128, tps, N, N], f16, tag="rowt27")

                    for tp in range(tps):
                        t = st * tps + tp
                        rows = slice(t * PT, (t + 1) * PT)
                        # --- load f32, natural layout [s, j, d] ---
                        nat = nat_pool.tile([128, N, D], f32)
                        if trmode == "f32":
                            nc.sync.dma_start(out=nat[:, 0, :], in_=dense[rows, :])
                            nc.sync.dma_start(
                                out=nat[:, 1:N, :],
                                in_=sparse[:, rows, :].rearrange("s b d -> b s d"),
                            )
                            trin = nat
                        else:
                            # SWDGE cast-DMA load straight to f16
                            nat16 = nat_pool.tile([128, N, D], f16, tag="nat16")
                            nc.gpsimd.dma_start(out=nat16[:, 0, :], in_=dense[rows, :])
                            nc.gpsimd.dma_start(
                                out=nat16[:, 1:N, :],
                                in_=sparse[:, rows, :].rearrange("s b d -> b s d"),
                            )
                            trin = nat16
                        if flatten in ("rowq", "rect", "stream") and upto in ("flat", "full"):
                            rq_eng = nc.gpsimd if rowq_eng == "gpsimd" else nc.sync
                            rq_eng.dma_start(out=rowq[:, tp, 0:D], in_=dense[rows, :])
                        if upto == "load":
                            continue

                        # --- TensorE transpose of each feature slab; for f32
                        # input the f32->f16 cast happens on the PSUM copy ---
                        xt = xt_pool.tile([128, N, PT], f16)
                        for _xr in range(xt_reps):
                            for k in range(7):  # 4-slab packs: 6*4 + 3
                                j0 = 4 * k
                                nj = min(4, N - j0)
                                pt_ = psumt_pool.tile([128, 4, PT], id_dt, tag="pt")
                                for jj in range(nj):
                                    nc.tensor.transpose(
                                        pt_[:, jj, :], trin[:, j0 + jj, :], ident
                                    )
                                cp = nc.vector.tensor_copy if k % 2 == 0 else nc.scalar.copy
                                cp(out=xt[:, j0 : j0 + nj, :], in_=pt_[:, 0:nj, :])

                        # --- Gram matmuls ---
                        if upto == "xt":
                            continue
                        if flatten == "rect" and upto != "mm":
                            gcol = gcol_pool.tile([128, gpt, N], gdt, tag="gct")
                        elif flatten == "stream" and upto != "mm":
                            gcol = gcol_pool.tile([128, N, gpt], f16, tag="gcs")
                        if upto == "mm" and mm_cols > 1:
                            # timing probe: same MM count, rhs widened to
                            # mm_cols samples (27*mm_cols free cols)
                            gpr_w = 512 // (N * mm_cols)
                            for r in range(gpt // gpr_w + (1 if gpt % gpr_w else 0)):
                                qs = list(range(r * gpr_w, min((r + 1) * gpr_w, gpt)))
                                psw = psum_pool.tile([128, len(qs), N * mm_cols], f32, tag="psw")
                                for qi, g_local in enumerate(qs):
                                    for c in range(4):
                                        s_local = 32 * c + g_local
                                        s0 = min(s_local, PT - mm_cols)
                                        nc.tensor.matmul(
                                            out=psw[32 * c : 32 * c + N, qi, :],
                                            lhsT=xt[:, :, s_local],
                                            rhs=xt[:, :, s0 : s0 + mm_cols],
                                            start=True,
                                            stop=True,
                                            tile_position=(0, 32 * c),
                                        )
                            continue
                        dr_cp = nc.vector.tensor_copy if drain_eng == "dve" else nc.scalar.copy
                        for r in range(rpt):
                            if gram_layout == "jq":
                                ps = psum_pool.tile([128, N, GPR], f32)
                            else:
                                ps = psum_pool.tile([128, GPR, N], f32)
                            for q in range(GPR):
                                g_local = r * GPR + q
                                for c in range(4):
                                    s_local = 32 * c + g_local
                                    lhs_s = 0 if shared_lhsT else s_local
                                    out_ap = (
                                        ps[32 * c : 32 * c + N, :, q]
                                        if gram_layout == "jq"
                                        else ps[32 * c : 32 * c + N, q, :]
                                    )
                                    nc.tensor.matmul(
                                        out=out_ap,
                                        lhsT=xt[:, :, lhs_s],
                                        rhs=xt[:, :, s_local],
                                        start=True,
                                        stop=True,
                                        tile_position=(0, 32 * c),
                                    )
                            if upto == "mm":
                                continue
                            off = r * GPR
                            # full-width copy: partitions (c,i) all at once;
                            # lanes 32c+27..32c+31 carry garbage, never read.
                            if flatten == "rect":
                                dr_cp(
                                    out=gcol[:, off : off + GPR, :], in_=ps[:, :, :]
                                )
                            elif flatten == "stream":
                                # (j, g) free layout, g innermost for the
                                # 32x32 stream transpose
                                if gram_layout == "jq":
                                    dr_cp(
                                        out=gcol[:, :, off : off + GPR],
                                        in_=ps[:, :, :],
                                    )
                                else:
                                    dr_cp(
                                        out=gcol[:, :, off : off + GPR],
                                        in_=ps[:, :, :].rearrange("p q j -> p j q"),
                                    )
                            else:
                                dr_cp(
                                    out=gcol[:, off : off + GPR, tp, :],
                                    in_=ps[:, :, :],
                                )

                        # --- stream transpose: 32x32 block transpose on DVE
                        # lands samples on partitions: strans[32c+g, tp, j, i]
                        if flatten == "stream" and upto in ("flat", "full"):
                            nc.vector.transpose(
                                out=strans[:, tp, :, :], in_=gcol[:, :, :]
                            )

                        # --- rect gather: partition transpose (c,i)->(c,g)
                        # per c-block, evenly spread reads ---
                        if flatten == "rect" and upto in ("flat", "full"):
                            for c in range(4):
                                geng = nc.sync if (tp * 4 + c) % 2 else nc.scalar
                                geng.dma_start(
                                    out=rowt27[32 * c : 32 * c + 32, tp, :, :],
                                    in_=gcol[32 * c : 32 * c + N, :, :].rearrange(
                                        "i g j -> g i j"
                                    ),
                                )

                    # --- flatten: tri row i of every sample; read partition
                    # 32c+i (stride-32 partition dim), free (g, tp, j) ---
                    if upto in ("load", "xt", "mm", "gram"):
                        continue
                    if flatten == "stream":
                        # tril-compact strans -> rowq with f16->f32 cast
                        for i in range(N):
                            toff = D + i * (i + 1) // 2
                            cpc = nc.vector.tensor_copy if i % 2 else nc.scalar.copy
                            cpc(
                                out=rowq[:, :, toff : toff + i + 1],
                                in_=strans[:, :, 0 : i + 1, i],
                            )
                        if upto == "full":
                            for tp in range(tps):
                                t = st * tps + tp
                                seng = nc.scalar if tp % store_split == 0 else nc.sync
                                seng.dma_start(
                                    out=out[t * PT : (t + 1) * PT, :],
                                    in_=rowq[:, tp, :],
                                )
                    elif flatten == "rect":
                        # tril-compact rowt27 -> rowq with f16->f32 cast
                        for i in range(N):
                            toff = D + i * (i + 1) // 2
                            cpc = nc.vector.tensor_copy if i % 2 else nc.scalar.copy
                            cpc(
                                out=rowq[:, :, toff : toff + i + 1],
                                in_=rowt27[:, :, i, 0 : i + 1],
                            )
                        if upto == "full":
                            for tp in range(tps):
                                t = st * tps + tp
                                seng = nc.scalar if tp % store_split == 0 else nc.sync
                                seng.dma_start(
                                    out=out[t * PT : (t + 1) * PT, :],
                                    in_=rowq[:, tp, :],
                                )
                    elif flatten == "hbm":
                        ost = out[st * tps * PT : (st + 1) * tps * PT, :].rearrange(
                            "(tp cg) w -> cg tp w", tp=tps
                        )
                        for i in range(N):
                            toff = D + i * (i + 1) // 2
                            nc.scalar.dma_start(
                                out=ost[:, :, toff : toff + i + 1],
                                in_=gcol[i : 97 + i + 1 : 32, :, :, 0 : i + 1],
                            )
                    else:
                        for i in range(N):
                            toff = D + i * (i + 1) // 2
                            eng = nc.scalar if i % flat_split == 0 else nc.sync
                            if gcol_f16:
                                toff_t = toff - D
                                eng.dma_start(
                                    out=rowt[:, :, toff_t : toff_t + i + 1],
                                    in_=gcol[i : 97 + i + 1 : 32, :, :, 0 : i + 1],
                                )
                            else:
                                eng.dma_start(
                                    out=rowq[:, :, toff : toff + i + 1],
                                    in_=gcol[i : 97 + i + 1 : 32, :, :, 0 : i + 1],
                                )
                        if gcol_f16:
                            # cast tri f16 -> f32 into rowq (DVE/ACT split)
                            for tp in range(tps):
                                cpc = nc.vector.tensor_copy if tp % 2 else nc.scalar.copy
                                cpc(
                                    out=rowq[:, tp, D:W],
                                    in_=rowt[:, tp, :],
                                )
                        if upto == "full":
                            if big_store:
                                ost = out[
                                    st * tps * PT : (st + 1) * tps * PT, :
                                ].rearrange("(tp cg) w -> cg tp w", tp=tps)
                                nc.scalar.dma_start(out=ost, in_=rowq[:, :, :])
                            else:
                                for tp in range(tps):
                                    t = st * tps + tp
                                    seng = nc.scalar if tp % store_split == 0 else nc.sync
                                    seng.dma_start(
                                        out=out[t * PT : (t + 1) * PT, :],
                                        in_=rowq[:, tp, :],
                                    )

    nc.compile()
    return nc


def build_kernel_v2(
    b_core: int = BC,
    reps: int = 1,
    *,
    tps: int = 4,
    burst: bool = True,  # per-supertile phases: all loads, all transposes, all MMs
    jq: bool = True,  # strided MM out -> contiguous gram drain
    rowq_eng: str = "sync",
    stores: str = "scalar",  # "scalar" | "sync" | "split"
    nat_bufs: int = 8,
    psum_bufs: int = 3,
    psumt_bufs: int = 4,
    heater: int = 0,  # junk 128-col MMs appended per supertile to keep HAM warm
    drain_eng: str = "act",
    upto: str = "full",
):
    """Stream-flatten DotInteraction, restructured so PE work is contiguous:
    per supertile all loads are issued, then all 27*tps transposes, then all
    128*tps Gram matmuls -- one long PE burst per supertile instead of 2*tps
    short bursts, so the HAM clock gate stays at full rate."""
    nc = bacc.Bacc("TRN2", target_bir_lowering=False, debug=False)
    dense = nc.dram_tensor("dense", [b_core, D], f32, kind="ExternalInput").ap()
    sparse = nc.dram_tensor("sparse", [S, b_core, D], f32, kind="ExternalInput").ap()
    out = nc.dram_tensor("out", [b_core, W], f32, kind="ExternalOutput").ap()

    t_total = b_core // PT
    gpt = PT // 4  # 32 sample groups per tile (4-way col tiling)
    rpt = gpt // GPR  # psum rounds per tile
    tps = min(tps, t_total)
    n_super = t_total // tps

    with tile.TileContext(nc) as tc:
        with (
            tc.tile_pool(name="singles", bufs=1) as singles,
            tc.tile_pool(name="nat", bufs=nat_bufs) as nat_pool,
            tc.tile_pool(name="xt", bufs=tps) as xt_pool,
            tc.tile_pool(name="gcol", bufs=4) as gcol_pool,
            tc.tile_pool(name="row", bufs=2) as row_pool,
            tc.tile_pool(name="psum", bufs=psum_bufs, space="PSUM") as psum_pool,
            tc.tile_pool(name="psumt", bufs=psumt_bufs, space="PSUM") as psumt_pool,
            tc.tile_pool(name="hps", bufs=1, space="PSUM") as hps_pool,
        ):
            ident = singles.tile([128, 128], f16, name="ident")
            make_identity(nc, ident)
            if heater:
                hps = hps_pool.tile([128, 128], f32)

            for _rep in range(reps):
                for st in range(n_super):
                    rowq = row_pool.tile([128, tps, W], f32)
                    strans = row_pool.tile([128, tps, N, 32], f16, tag="strans")
                    nats = []
                    xts = []

                    # --- phase 1: loads (SWDGE cast f32->f16) ---
                    for tp in range(tps):
                        t = st * tps + tp
                        rows = slice(t * PT, (t + 1) * PT)
                        nat16 = nat_pool.tile([128, N, D], f16, tag="nat16")
                        nc.gpsimd.dma_start(out=nat16[:, 0, :], in_=dense[rows, :])
                        nc.gpsimd.dma_start(
                            out=nat16[:, 1:N, :],
                            in_=sparse[:, rows, :].rearrange("s b d -> b s d"),
                        )
                        nats.append(nat16)
                        if upto == "full":
                            rq_eng = nc.gpsimd if rowq_eng == "gpsimd" else nc.sync
                            rq_eng.dma_start(out=rowq[:, tp, 0:D], in_=dense[rows, :])

                    # --- phase 2: PE transposes (27 per tile, contiguous) ---
                    for tp in range(tps):
                        xt = xt_pool.tile([128, N, PT], f16)
                        for k in range(7):
                            j0 = 4 * k
                            nj = min(4, N - j0)
                            pt_ = psumt_pool.tile([128, 4, PT], f16, tag="pt")
                            for jj in range(nj):
                                nc.tensor.transpose(
                                    pt_[:, jj, :], nats[tp][:, j0 + jj, :], ident
                                )
                            cp = nc.vector.tensor_copy if k % 2 == 0 else nc.scalar.copy
                            cp(out=xt[:, j0 : j0 + nj, :], in_=pt_[:, 0:nj, :])
                        xts.append(xt)

                    if upto == "xt":
                        continue

                    # --- phase 3: Gram MMs (128 per tile, contiguous burst) ---
                    dr_cp = nc.vector.tensor_copy if drain_eng == "dve" else nc.scalar.copy
                    for tp in range(tps):
                        xt = xts[tp]
                        gcol = gcol_pool.tile([128, N, gpt], f16, tag="gcs")
                        for r in range(rpt):
                            if jq:
                                ps = psum_pool.tile([128, N, GPR], f32)
                            else:
                                ps = psum_pool.tile([128, GPR, N], f32)
                            for q in range(GPR):
                                g_local = r * GPR + q
                                for c in range(4):
                                    s_local = 32 * c + g_local
                                    out_ap = (
                                        ps[32 * c : 32 * c + N, :, q]
                                        if jq
                                        else ps[32 * c : 32 * c + N, q, :]
                                    )
                                    nc.tensor.matmul(
                                        out=out_ap,
                                        lhsT=xt[:, :, s_local],
                                        rhs=xt[:, :, s_local],
                                        start=True,
                                        stop=True,
                                        tile_position=(0, 32 * c),
                                    )
                            off = r * GPR
                            if jq:
                                dr_cp(out=gcol[:, :, off : off + GPR], in_=ps[:, :, :])
                            else:
                                dr_cp(
                                    out=gcol[:, :, off : off + GPR],
                                    in_=ps[:, :, :].rearrange("p q j -> p j q"),
                                )
                        # 32x32 sample-block transpose: samples -> partitions
                        nc.vector.transpose(out=strans[:, tp, :, :], in_=gcol[:, :, :])

                    if heater:
                        for _h in range(heater):
                            nc.tensor.matmul(
                                out=hps[:, :],
                                lhsT=ident[:, :],
                                rhs=ident[:, :],
                                start=True,
                                stop=True,
                            )

                    if upto != "full":
                        continue

                    # --- flatten: tril-compact strans -> rowq (f16->f32) ---
                    for i in range(N):
                        toff = D + i * (i + 1) // 2
                        cpc = nc.vector.tensor_copy if i % 2 else nc.scalar.copy
                        cpc(
                            out=rowq[:, :, toff : toff + i + 1],
                            in_=strans[:, :, 0 : i + 1, i],
                        )
                    for tp in range(tps):
                        t = st * tps + tp
                        if stores == "scalar":
                            seng = nc.scalar
                        elif stores == "sync":
                            seng = nc.sync
                        else:
                            seng = nc.scalar if tp % 2 == 0 else nc.sync
                        seng.dma_start(
                            out=out[t * PT : (t + 1) * PT, :], in_=rowq[:, tp, :]
                        )

    nc.compile()
    return nc


_CACHE: dict = {}


def _get_nc():
    if "nc" not in _CACHE:
        _CACHE["nc"] = build_kernel(BC)
    return _CACHE["nc"]


def kernel(dense_feature, sparse_stack, **run_kwargs):
    dense_feature = np.asarray(dense_feature, dtype=np.float32)
    sparse_stack = np.asarray(sparse_stack, dtype=np.float32)
    assert dense_feature.shape == (B, D)
    assert sparse_stack.shape == (S, B, D)

    nc = run_kwargs.pop("nc", None) or _get_nc()
    in_maps = []
    for ci in range(NCORES):
        sl = slice(ci * BC, (ci + 1) * BC)
        in_maps.append(
            {
                "dense": np.ascontiguousarray(dense_feature[sl]),
                "sparse": np.ascontiguousarray(sparse_stack[:, sl, :]),
            }
        )
    res = bass_utils.run_bass_kernel_spmd(
        nc, in_maps, core_ids=list(range(NCORES)), **run_kwargs
    )
    out = np.concatenate([r["out"] for r in res.results], axis=0)
    if run_kwargs:
        _CACHE["last_result"] = res
    return out



# revision 21
# speedup vs baseline: 1.0577x; 1.0577x over previous
"""DLRM DotInteraction kernel for Trainium2 (Bass/Tile), 8-core data parallel.

Problem: dense_feature [B=16384, D=128] f32, sparse_stack [S=26, B, D] f32.
cat = [dense; sparse] per sample -> [B, N=27, D]; G_b = cat_b @ cat_b^T;
out = [dense | tril(G_b) (378 vals, row-major incl diag)] -> [B, 506] f32.

Per core (B_c = 2048 samples = 16 tiles of 128), default config ("stream"):
  1. SWDGE cast-DMA loads f32 HBM -> f16 SBUF, natural layout nat[s, j, d].
  2. TensorE f16 transpose of each feature slab -> PSUM, DVE/ACT copy to
     SBUF xt[d, j, s].
  3. TensorE Gram per sample: 4 col-tiled matmuls per group (tile_position
     (0,32c)), K=128, M=N=27, f16 in, f32 PSUM [32c+i, q, j].
  4. Full-width ACT copy PSUM -> gcol[(c,i) part, (j, g)] f16 per tile.
  5. DVE 32x32 StreamTranspose swaps (i<->g) within 32-blocks: lands
     samples on partitions (strans[(c,g) part, tp, j, i]) with NO DMA.
  6. Tril-compaction: 27 engine copies per supertile (f16->f32 cast fused),
     then one contiguous 259 KB store per tile (split over both HWDGE rings).
"""

import numpy as np

import concourse.bacc as bacc
import concourse.mybir as mybir
import concourse.tile as tile
from concourse import bass_utils
from concourse.masks import make_identity

B = 16384
D = 128
S = 26
N = S + 1  # 27
NCORES = 8
BC = B // NCORES  # 2048 samples per core
PT = 128  # samples per sbuf tile
GPR = 16  # groups per psum round
TRI = N * (N + 1) // 2  # 378
W = D + TRI  # 506
TPS = 8  # tiles per supertile

f32 = mybir.dt.float32
f16 = mybir.dt.float16


def build_kernel(
    b_core: int = BC,
    reps: int = 1,
    *,
    tps: int = 4,
    flatten: str = "stream",  # "stream" | "rowq" | "hbm" (see below)
    trmode: str = "f16",  # "f32" (PE f32 transpose) | "f16" (cast then f16 transpose)
    nat_bufs: int = 4,
    xt_bufs: int = 4,
    gcol_bufs: int = 4,
    upto: str = "full",  # "load" | "xt" | "gram" | "flat" | "full" (bench ablation)
    flat_split: int = 2,  # flatten DMA queue split: i % flat_split == 0 -> scalar
    store_split: int = 2,  # store queue split: tp % store_split == 0 -> scalar
    shared_lhsT: bool = False,  # timing probe: all gram matmuls share one lhsT
    dense_via_rowq: bool = False,  # f32 mode: dense loaded once (into rowq),
    # j=0 transpose reads rowq; saves the duplicate dense HBM read
    row_bufs: int = 2,
    gcol_f16: bool = False,  # gcol+tri in f16, engine-cast to f32 before store
    big_store: bool = False,  # one store DMA per supertile instead of per tile
    psum_bufs: int = 4,
    psumt_bufs: int = 4,
    gram_layout: str = "qj",  # "qj" (baseline) | "jq" (strided MM out; contiguous drain)
    drain_eng: str = "act",  # gram psum->gcol drain engine: "act" | "dve"
    rowq_eng: str = "sync",  # rowq dense-load DMA queue: "sync" | "gpsimd"
    mm_cols: int = 1,  # timing probe (mm mode only): rhs spans this many samples
    xt_reps: int = 1,  # timing probe: issue the transpose stage this many times
):
    nc = bacc.Bacc("TRN2", target_bir_lowering=False, debug=False)
    dense = nc.dram_tensor("dense", [b_core, D], f32, kind="ExternalInput").ap()
    sparse = nc.dram_tensor("sparse", [S, b_core, D], f32, kind="ExternalInput").ap()
    out = nc.dram_tensor("out", [b_core, W], f32, kind="ExternalOutput").ap()

    t_total = b_core // PT
    gpt = PT // 4  # 32 groups per tile
    rpt = gpt // GPR  # psum rounds per tile
    tps = min(tps, t_total)
    n_super = t_total // tps

    with tile.TileContext(nc) as tc:
        with (
            tc.tile_pool(name="singles", bufs=1) as singles,
            tc.tile_pool(name="nat", bufs=nat_bufs) as nat_pool,
            tc.tile_pool(name="xt", bufs=xt_bufs) as xt_pool,
            tc.tile_pool(name="gcol", bufs=gcol_bufs) as gcol_pool,
            tc.tile_pool(name="row", bufs=row_bufs) as row_pool,
            tc.tile_pool(name="psum", bufs=psum_bufs, space="PSUM") as psum_pool,
            tc.tile_pool(name="psumt", bufs=psumt_bufs, space="PSUM") as psumt_pool,
        ):
            id_dt = f32 if trmode == "f32" else f16
            ident = singles.tile([128, 128], id_dt, name="ident")
            make_identity(nc, ident)

            for _rep in range(reps):
                if flatten == "hbm":
                    # dense passthrough: single HBM->HBM DMA
                    nc.scalar.dma_start(out=out[:, 0:D], in_=dense[:, :])
                for st in range(n_super):
                    # gcol[32c+i, g, tp, j] = Gram[i,j] of sample 32c+g in
                    # tile tp of this supertile.
                    gdt = f16 if gcol_f16 else f32
                    if flatten not in ("rect", "stream") and upto not in ("load", "xt", "mm"):
                        gcol = gcol_pool.tile([128, gpt, tps, N], gdt)
                    if flatten == "stream" and upto in ("flat", "full"):
                        rowq = row_pool.tile([128, tps, W], f32)
                        strans = row_pool.tile([128, tps, N, 32], f16, tag="strans")
                    elif flatten == "rowq":
                        rowq = row_pool.tile([128, tps, W], f32)
                        if gcol_f16:
                            rowt = row_pool.tile([128, tps, TRI], f16, tag="rowt")
                    elif flatten == "rect":
                        rowq = row_pool.tile([128, tps, W], f32)
                        rowt27 = row_pool.tile([128, tps, N, N], f16, tag="rowt27")

                    for tp in range(tps):
                        t = st * tps + tp
                        rows = slice(t * PT, (t + 1) * PT)
                        # --- load f32, natural layout [s, j, d] ---
                        nat = nat_pool.tile([128, N, D], f32)
                        if trmode == "f32":
                            nc.sync.dma_start(out=nat[:, 0, :], in_=dense[rows, :])
                            nc.sync.dma_start(
                                out=nat[:, 1:N, :],
                                in_=sparse[:, rows, :].rearrange("s b d -> b s d"),
                            )
                            trin = nat
                        else:
                            # SWDGE cast-DMA load straight to f16
                            nat16 = nat_pool.tile([128, N, D], f16, tag="nat16")
                            nc.gpsimd.dma_start(out=nat16[:, 0, :], in_=dense[rows, :])
                            nc.gpsimd.dma_start(
                                out=nat16[:, 1:N, :],
                                in_=sparse[:, rows, :].rearrange("s b d -> b s d"),
                            )
                            trin = nat16
                        if flatten in ("rowq", "rect", "stream") and upto in ("flat", "full"):
                            rq_eng = nc.gpsimd if rowq_eng == "gpsimd" else nc.sync
                            rq_eng.dma_start(out=rowq[:, tp, 0:D], in_=dense[rows, :])
                        if upto == "load":
                            continue

                        # --- TensorE transpose of each feature slab; for f32
                        # input the f32->f16 cast happens on the PSUM copy ---
                        xt = xt_pool.tile([128, N, PT], f16)
                        for _xr in range(xt_reps):
                            for k in range(7):  # 4-slab packs: 6*4 + 3
                                j0 = 4 * k
                                nj = min(4, N - j0)
                                pt_ = psumt_pool.tile([128, 4, PT], id_dt, tag="pt")
                                for jj in range(nj):
                                    nc.tensor.transpose(
                                        pt_[:, jj, :], trin[:, j0 + jj, :], ident
                                    )
                                cp = nc.vector.tensor_copy if k % 2 == 0 else nc.scalar.copy
                                cp(out=xt[:, j0 : j0 + nj, :], in_=pt_[:, 0:nj, :])

                        # --- Gram matmuls ---
                        if upto == "xt":
                            continue
                        if flatten == "rect" and upto != "mm":
                            gcol = gcol_pool.tile([128, gpt, N], gdt, tag="gct")
                        elif flatten == "stream" and upto != "mm":
                            gcol = gcol_pool.tile([128, N, gpt], f16, tag="gcs")
                        if upto == "mm" and mm_cols > 1:
                            # timing probe: same MM count, rhs widened to
                            # mm_cols samples (27*mm_cols free cols)
                            gpr_w = 512 // (N * mm_cols)
                            for r in range(gpt // gpr_w + (1 if gpt % gpr_w else 0)):
                                qs = list(range(r * gpr_w, min((r + 1) * gpr_w, gpt)))
                                psw = psum_pool.tile([128, len(qs), N * mm_cols], f32, tag="psw")
                                for qi, g_local in enumerate(qs):
                                    for c in range(4):
                                        s_local = 32 * c + g_local
                                        s0 = min(s_local, PT - mm_cols)
                                        nc.tensor.matmul(
                                            out=psw[32 * c : 32 * c + N, qi, :],
                                            lhsT=xt[:, :, s_local],
                                            rhs=xt[:, :, s0 : s0 + mm_cols],
                                            start=True,
                                            stop=True,
                                            tile_position=(0, 32 * c),
                                        )
                            continue
                        dr_cp = nc.vector.tensor_copy if drain_eng == "dve" else nc.scalar.copy
                        for r in range(rpt):
                            if gram_layout == "jq":
                                ps = psum_pool.tile([128, N, GPR], f32)
                            else:
                                ps = psum_pool.tile([128, GPR, N], f32)
                            for q in range(GPR):
                                g_local = r * GPR + q
                                for c in range(4):
                                    s_local = 32 * c + g_local
                                    lhs_s = 0 if shared_lhsT else s_local
                                    out_ap = (
                                        ps[32 * c : 32 * c + N, :, q]
                                        if gram_layout == "jq"
                                        else ps[32 * c : 32 * c + N, q, :]
                                    )
                                    nc.tensor.matmul(
                                        out=out_ap,
                                        lhsT=xt[:, :, lhs_s],
                                        rhs=xt[:, :, s_local],
                                        start=True,
                                        stop=True,
                                        tile_position=(0, 32 * c),
                                    )
                            if upto == "mm":
                                continue
                            off = r * GPR
                            # full-width copy: partitions (c,i) all at once;
                            # lanes 32c+27..32c+31 carry garbage, never read.
                            if flatten == "rect":
                                dr_cp(
                                    out=gcol[:, off : off + GPR, :], in_=ps[:, :, :]
                                )
                            elif flatten == "stream":
                                # (j, g) free layout, g innermost for the
                                # 32x32 stream transpose
                                if gram_layout == "jq":
                                    dr_cp(
                                        out=gcol[:, :, off : off + GPR],
                                        in_=ps[:, :, :],
                                    )
                                else:
                                    dr_cp(
                                        out=gcol[:, :, off : off + GPR],
                                        in_=ps[:, :, :].rearrange("p q j -> p j q"),
                                    )
                            else:
                                dr_cp(
                                    out=gcol[:, off : off + GPR, tp, :],
                                    in_=ps[:, :, :],
                                )

                        # --- stream transpose: 32x32 block transpose on DVE
                        # lands samples on partitions: strans[32c+g, tp, j, i]
                        if flatten == "stream" and upto in ("flat", "full"):
                            nc.vector.transpose(
                                out=strans[:, tp, :, :], in_=gcol[:, :, :]
                            )

                        # --- rect gather: partition transpose (c,i)->(c,g)
                        # per c-block, evenly spread reads ---
                        if flatten == "rect" and upto in ("flat", "full"):
                            for c in range(4):
                                geng = nc.sync if (tp * 4 + c) % 2 else nc.scalar
                                geng.dma_start(
                                    out=rowt27[32 * c : 32 * c + 32, tp, :, :],
                                    in_=gcol[32 * c : 32 * c + N, :, :].rearrange(
                                        "i g j -> g i j"
                                    ),
                                )

                    # --- flatten: tri row i of every sample; read partition
                    # 32c+i (stride-32 partition dim), free (g, tp, j) ---
                    if upto in ("load", "xt", "mm", "gram"):
                        continue
                    if flatten == "stream":
                        # tril-compact strans -> rowq with f16->f32 cast
                        for i in range(N):
                            toff = D + i * (i + 1) // 2
                            cpc = nc.vector.tensor_copy if i % 2 else nc.scalar.copy
                            cpc(
                                out=rowq[:, :, toff : toff + i + 1],
                                in_=strans[:, :, 0 : i + 1, i],
                            )
                        if upto == "full":
                            for tp in range(tps):
                                t = st * tps + tp
                                seng = nc.scalar if tp % store_split == 0 else nc.sync
                                seng.dma_start(
                                    out=out[t * PT : (t + 1) * PT, :],
                                    in_=rowq[:, tp, :],
                                )
                    elif flatten == "rect":
                        # tril-compact rowt27 -> rowq with f16->f32 cast
                        for i in range(N):
                            toff = D + i * (i + 1) // 2
                            cpc = nc.vector.tensor_copy if i % 2 else nc.scalar.copy
                            cpc(
                                out=rowq[:, :, toff : toff + i + 1],
                                in_=rowt27[:, :, i, 0 : i + 1],
                            )
                        if upto == "full":
                            for tp in range(tps):
                                t = st * tps + tp
                                seng = nc.scalar if tp % store_split == 0 else nc.sync
                                seng.dma_start(
                                    out=out[t * PT : (t + 1) * PT, :],
                                    in_=rowq[:, tp, :],
                                )
                    elif flatten == "hbm":
                        ost = out[st * tps * PT : (st + 1) * tps * PT, :].rearrange(
                            "(tp cg) w -> cg tp w", tp=tps
                        )
                        for i in range(N):
                            toff = D + i * (i + 1) // 2
                            nc.scalar.dma_start(
                                out=ost[:, :, toff : toff + i + 1],
                                in_=gcol[i : 97 + i + 1 : 32, :, :, 0 : i + 1],
                            )
                    else:
                        for i in range(N):
                            toff = D + i * (i + 1) // 2
                            eng = nc.scalar if i % flat_split == 0 else nc.sync
                            if gcol_f16:
                                toff_t = toff - D
                                eng.dma_start(
                                    out=rowt[:, :, toff_t : toff_t + i + 1],
                                    in_=gcol[i : 97 + i + 1 : 32, :, :, 0 : i + 1],
                                )
                            else:
                                eng.dma_start(
                                    out=rowq[:, :, toff : toff + i + 1],
                                    in_=gcol[i : 97 + i + 1 : 32, :, :, 0 : i + 1],
                                )
                        if gcol_f16:
                            # cast tri f16 -> f32 into rowq (DVE/ACT split)
                            for tp in range(tps):
                                cpc = nc.vector.tensor_copy if tp % 2 else nc.scalar.copy
                                cpc(
                                    out=rowq[:, tp, D:W],
                                    in_=rowt[:, tp, :],
                                )
                        if upto == "full":
                            if big_store:
                                ost = out[
                                    st * tps * PT : (st + 1) * tps * PT, :
                                ].rearrange("(tp cg) w -> cg tp w", tp=tps)
                                nc.scalar.dma_start(out=ost, in_=rowq[:, :, :])
                            else:
                                for tp in range(tps):
                                    t = st * tps + tp
                                    seng = nc.scalar if tp % store_split == 0 else nc.sync
                                    seng.dma_start(
                                        out=out[t * PT : (t + 1) * PT, :],
                                        in_=rowq[:, tp, :],
                                    )

    nc.compile()
    return nc


def build_kernel_v2(
    b_core: int = BC,
    reps: int = 1,
    *,
    tps: int = 4,
    burst: bool = True,  # per-supertile phases: all loads, all transposes, all MMs
    jq: bool = True,  # strided MM out -> contiguous gram drain
    rowq_eng: str = "sync",
    stores: str = "scalar",  # "scalar" | "sync" | "split"
    nat_bufs: int = 8,
    psum_bufs: int = 3,
    psumt_bufs: int = 4,
    heater: int = 0,  # junk 128-col MMs appended per supertile to keep HAM warm
    drain_eng: str = "act",
    upto: str = "full",
    pack4: bool = False,  # 4 samples per MM (M=128, 108 cols): 32 MMs/tile not 128
    xt_dve: int = 4,  # transpose-drain packs routed to DVE (rest ACT)
):
    """Stream-flatten DotInteraction, restructured so PE work is contiguous:
    per supertile all loads are issued, then all 27*tps transposes, then all
    128*tps Gram matmuls -- one long PE burst per supertile instead of 2*tps
    short bursts, so the HAM clock gate stays at full rate."""
    nc = bacc.Bacc("TRN2", target_bir_lowering=False, debug=False)
    dense = nc.dram_tensor("dense", [b_core, D], f32, kind="ExternalInput").ap()
    sparse = nc.dram_tensor("sparse", [S, b_core, D], f32, kind="ExternalInput").ap()
    out = nc.dram_tensor("out", [b_core, W], f32, kind="ExternalOutput").ap()

    t_total = b_core // PT
    gpt = PT // 4  # 32 sample groups per tile (4-way col tiling)
    rpt = gpt // GPR  # psum rounds per tile
    tps = min(tps, t_total)
    n_super = t_total // tps

    with tile.TileContext(nc) as tc:
        with (
            tc.tile_pool(name="singles", bufs=1) as singles,
            tc.tile_pool(name="nat", bufs=nat_bufs) as nat_pool,
            tc.tile_pool(name="xt", bufs=tps) as xt_pool,
            tc.tile_pool(name="gcol", bufs=4) as gcol_pool,
            tc.tile_pool(name="row", bufs=2) as row_pool,
            tc.tile_pool(name="psum", bufs=psum_bufs, space="PSUM") as psum_pool,
            tc.tile_pool(name="psumt", bufs=psumt_bufs, space="PSUM") as psumt_pool,
            tc.tile_pool(name="hps", bufs=1, space="PSUM") as hps_pool,
        ):
            ident = singles.tile([128, 128], f16, name="ident")
            make_identity(nc, ident)
            if heater:
                hps = hps_pool.tile([128, 128], f32)

            for _rep in range(reps):
                for st in range(n_super):
                    rowq = row_pool.tile([128, tps, W], f32)
                    strans = row_pool.tile([128, tps, N, 32], f16, tag="strans")
                    nats = []
                    xts = []

                    # --- phase 1: loads (SWDGE cast f32->f16) ---
                    for tp in range(tps):
                        t = st * tps + tp
                        rows = slice(t * PT, (t + 1) * PT)
                        nat16 = nat_pool.tile([128, N, D], f16, tag="nat16")
                        nc.gpsimd.dma_start(out=nat16[:, 0, :], in_=dense[rows, :])
                        nc.gpsimd.dma_start(
                            out=nat16[:, 1:N, :],
                            in_=sparse[:, rows, :].rearrange("s b d -> b s d"),
                        )
                        nats.append(nat16)
                        if upto == "full":
                            rq_eng = nc.gpsimd if rowq_eng == "gpsimd" else nc.sync
                            rq_eng.dma_start(out=rowq[:, tp, 0:D], in_=dense[rows, :])

                    # --- phase 2: PE transposes (27 per tile, contiguous) ---
                    for tp in range(tps):
                        nj_alloc = 32 if pack4 else N
                        xt = xt_pool.tile([128, nj_alloc, PT], f16)
                        for k in range(7):
                            j0 = 4 * k
                            nj = min(4, N - j0)
                            pt_ = psumt_pool.tile([128, 4, PT], f16, tag="pt")
                            for jj in range(nj):
                                nc.tensor.transpose(
                                    pt_[:, jj, :], nats[tp][:, j0 + jj, :], ident
                                )
                            cp = nc.vector.tensor_copy if k < xt_dve else nc.scalar.copy
                            cp(out=xt[:, j0 : j0 + nj, :], in_=pt_[:, 0:nj, :])
                            if pack4 and k == 1:
                                # fill pad feature rows 27..31 with (finite)
                                # copies of rows 0..4; products land in out
                                # cols/lanes that are never read
                                nc.vector.tensor_copy(
                                    out=xt[:, N : nj_alloc, :], in_=xt[:, 0:5, :]
                                )
                        xts.append(xt)

                    if upto == "xt":
                        continue

                    # --- phase 3: Gram MMs (128 per tile, contiguous burst) ---
                    dr_cp = nc.vector.tensor_copy if drain_eng == "dve" else nc.scalar.copy
                    for tp in range(tps):
                        xt = xts[tp]
                        gcol = gcol_pool.tile([128, N, gpt], f16, tag="gcs")
                        if pack4:
                            # 4 samples (g, g+32, g+64, g+96) per MM: lhsT is
                            # their 32-padded feature blocks (M=128), rhs their
                            # 27 features s-major (108 cols). Diagonal blocks
                            # land at psum cols q+4*(27a+j) -> per-block (j,q)
                            # contiguous after the stride-4 view.
                            for pj in range(4):
                                psj = psum_pool.tile([128, 2, 512], f32, tag="psj")
                                for rq in range(2):
                                    psv = psj[:, rq, :].rearrange("p (m q) -> p m q", q=4)
                                    for q in range(4):
                                        g = pj * 8 + rq * 4 + q
                                        nc.tensor.matmul(
                                            out=psv[:, 0:108, q],
                                            lhsT=xt[:, :, g::32].rearrange("d j s -> d s j"),
                                            rhs=xt[:, 0:N, g::32].rearrange("d j s -> d s j"),
                                            start=True,
                                            stop=True,
                                        )
                                goff = pj * 8
                                for a in range(4):
                                    cpd = nc.vector.tensor_copy if (pj + a) % 2 else nc.scalar.copy
                                    cpd(
                                        out=gcol[32 * a : 32 * a + 32, :, goff : goff + 8].rearrange(
                                            "p j (r q) -> p r j q", r=2
                                        ),
                                        in_=psj[32 * a : 32 * a + 32, :, 108 * a : 108 * a + 108].rearrange(
                                            "p r (j q) -> p r j q", q=4
                                        ),
                                    )
                            nc.vector.transpose(out=strans[:, tp, :, :], in_=gcol[:, :, :])
                            continue
                        for r in range(rpt):
                            if jq:
                                ps = psum_pool.tile([128, N, GPR], f32)
                            else:
                                ps = psum_pool.tile([128, GPR, N], f32)
                            for q in range(GPR):
                                g_local = r * GPR + q
                                for c in range(4):
                                    s_local = 32 * c + g_local
                                    out_ap = (
                                        ps[32 * c : 32 * c + N, :, q]
                                        if jq
                                        else ps[32 * c : 32 * c + N, q, :]
                                    )
                                    nc.tensor.matmul(
                                        out=out_ap,
                                        lhsT=xt[:, :, s_local],
                                        rhs=xt[:, :, s_local],
                                        start=True,
                                        stop=True,
                                        tile_position=(0, 32 * c),
                                    )
                            off = r * GPR
                            if jq:
                                dr_cp(out=gcol[:, :, off : off + GPR], in_=ps[:, :, :])
                            else:
                                dr_cp(
                                    out=gcol[:, :, off : off + GPR],
                                    in_=ps[:, :, :].rearrange("p q j -> p j q"),
                                )
                        # 32x32 sample-block transpose: samples -> partitions
                        nc.vector.transpose(out=strans[:, tp, :, :], in_=gcol[:, :, :])

                    if heater:
                        for _h in range(heater):
                            nc.tensor.matmul(
                                out=hps[:, :],
                                lhsT=ident[:, :],
                                rhs=ident[:, :],
                                start=True,
                                stop=True,
                            )

                    if upto != "full":
                        continue

                    # --- flatten: tril-compact strans -> rowq (f16->f32) ---
                    for i in range(N):
                        toff = D + i * (i + 1) // 2
                        cpc = nc.vector.tensor_copy if i % 2 else nc.scalar.copy
                        cpc(
                            out=rowq[:, :, toff : toff + i + 1],
                            in_=strans[:, :, 0 : i + 1, i],
                        )
                    for tp in range(tps):
                        t = st * tps + tp
                        if stores == "scalar":
                            seng = nc.scalar
                        elif stores == "sync":
                            seng = nc.sync
                        else:
                            seng = nc.scalar if tp % 2 == 0 else nc.sync
                        seng.dma_start(
                            out=out[t * PT : (t + 1) * PT, :], in_=rowq[:, tp, :]
                        )

    nc.compile()
    return nc


_CACHE: dict = {}


def _get_nc():
    if "nc" not in _CACHE:
        _CACHE["nc"] = build_kernel(BC)
    return _CACHE["nc"]


def kernel(dense_feature, sparse_stack, **run_kwargs):
    dense_feature = np.asarray(dense_feature, dtype=np.float32)
    sparse_stack = np.asarray(sparse_stack, dtype=np.float32)
    assert dense_feature.shape == (B, D)
    assert sparse_stack.shape == (S, B, D)

    nc = run_kwargs.pop("nc", None) or _get_nc()
    in_maps = []
    for ci in range(NCORES):
        sl = slice(ci * BC, (ci + 1) * BC)
        in_maps.append(
            {
                "dense": np.ascontiguousarray(dense_feature[sl]),
                "sparse": np.ascontiguousarray(sparse_stack[:, sl, :]),
            }
        )
    res = bass_utils.run_bass_kernel_spmd(
        nc, in_maps, core_ids=list(range(NCORES)), **run_kwargs
    )
    out = np.concatenate([r["out"] for r in res.results], axis=0)
    if run_kwargs:
        _CACHE["last_result"] = res
    return out

